# revision 54
# baseline (speedup 1.0000x reference)
"""DeBERTa-style 12-layer transformer on 8 TRN2 NeuronCores.

Sharding: data-parallel over batch (B=8 -> 1 sequence per core, no
collectives). Weights are host-prepped (transposed/tiled/fp16) and
replicated per core. Relative-position tables are expanded on host into
per-layer T1/T2 tables; the (q,k)-dependent gather is done on device via
matmul + a strided "skew" DMA read from a DRAM scratch buffer.

v2 layout/scheduling notes vs the original baseline:
- all weights are loaded once per layer in a handful of large DMAs
  (host pre-arranges each matrix so every DMA is a natural 2D slice)
- positional C blocks for all 12 heads are produced before the VG
  projection so the DRAM round trip hides behind GEMM work
- the per-head score assembly happens inside PSUM: content scores are
  matmul'd, the skewed c2p tiles are transpose-accumulated, p2c tiles are
  added via identity matmul, and the attention mask rides in as the bias
  operand of the exp activation
- softmax denominator comes from a ones-column appended to v (65-row ctx
  accumulator), normalization happens on the transposed ctx tile
- elementwise work is rotated across DVE / Pool / Act engines
"""

import itertools
import math
import numpy as np
import ml_dtypes

import concourse.bacc as bacc
import concourse.bass as bass
import concourse.mybir as mybir
from concourse import tile
from concourse.bass_utils import run_bass_kernel_spmd
from concourse.masks import make_identity

BF = ml_dtypes.bfloat16
F16 = np.float16
bf16 = mybir.dt.bfloat16
fp16 = mybir.dt.float16
f32 = mybir.dt.float32
fp8 = mybir.dt.float8e4

V = 32768; H = 768; NH = 12; D = 64; L = 12; FI = 2048
S = 512; B = 8; BK = 32; MAXP = 512; EPS = 1e-7
SCALE = 1.0 / math.sqrt(3 * D)
NQT = S // 128      # 4 token tiles
NHT = H // 128      # 6 hidden tiles
WEXP = 640          # C-block width (per-tile expansion window)
CBLK = 128 * WEXP
MASK_NEG = -60000.0
AF = mybir.ActivationFunctionType
ALU = mybir.AluOpType


# ---------------------------------------------------------------- host math
def _beta_delta():
    """bucket(delta)+31 for delta in [-511, 511], indexed by delta+511."""
    delta = np.arange(-(S - 1), S)
    sign = np.sign(delta)
    mid = BK // 2
    abs_pos = np.where((delta < mid) & (delta > -mid), mid - 1,
                       np.minimum(np.abs(delta), MAXP - 1))
    log_pos = np.ceil(np.log(abs_pos / mid) / math.log((MAXP - 1) / mid)
                      * (mid - 1)).astype(np.int64) + mid
    bucket = np.where(abs_pos <= mid, delta, log_pos * sign).astype(np.int64)
    return bucket + BK - 1


def _ln_np(x):
    m = x.mean(-1, keepdims=True)
    v = x.var(-1, keepdims=True)
    return (x - m) / np.sqrt(v + EPS)


# ---------------------------------------------------------------- builder
def _build(n_layers):
    nc = bacc.Bacc("TRN2", target_bir_lowering=False, num_devices=B)

    # ---- dram inputs (host-prepped layouts) ----
    wqk = nc.dram_tensor("wqk", [n_layers, 128, NHT * 1536], fp16, kind="ExternalInput")
    wvg = nc.dram_tensor("wvg", [n_layers, 128, NHT * 1536], fp16, kind="ExternalInput")
    wout = nc.dram_tensor("wout", [n_layers, 128, NHT * 768], fp16, kind="ExternalInput")
    w1 = nc.dram_tensor("w1", [n_layers, 8, 128, NHT * 512], fp16, kind="ExternalInput")
    w2 = nc.dram_tensor("w2", [n_layers, 8, 128, 2 * 768], fp16, kind="ExternalInput")
    t12d = nc.dram_tensor("t12d", [n_layers, NH // 2, 128, 2048], fp16, kind="ExternalInput")
    bqkd = nc.dram_tensor("bqkd", [n_layers, 128, 12], f32, kind="ExternalInput")
    # bbd: [0:1536]=bvg, [1536:2304]=-rowsum(Wout), [2304:3072]=-rowsum(W2)
    bbd = nc.dram_tensor("bbd", [n_layers, 1, 3072], fp16, kind="ExternalInput")
    # bout pre-broadcast to all partitions on host
    bcd = nc.dram_tensor("bcd", [n_layers, 128, 768], fp16, kind="ExternalInput")
    x0d = nc.dram_tensor("x0d", [NQT, 128, H], f32, kind="ExternalInput")
    maskd = nc.dram_tensor("maskd", [128, NQT], f32, kind="ExternalInput")
    yd = nc.dram_tensor("yd", [NQT, 128, H], f32, kind="ExternalOutput")

    # dram scratch for positional C blocks: [par, tbl, head, bt, 128, WEXP]
    cd = nc.dram_tensor("cd", [2, 2, NH, NQT, 128, WEXP], fp16, kind="Internal")

    def cd_base(par, tbl, h):
        return (((par * 2 + tbl) * NH + h) * NQT) * CBLK

    with tile.TileContext(nc) as tc:
        import contextlib
        ctx = contextlib.ExitStack()
        with ctx:
            pp = ctx.enter_context(tc.tile_pool(name="persist", bufs=1))
            sb = ctx.enter_context(tc.tile_pool(name="work", bufs=2))
            wp = ctx.enter_context(tc.tile_pool(name="wts", bufs=1))
            ps_mm = ctx.enter_context(tc.tile_pool(name="psmm", bufs=2, space="PSUM"))
            ps_aux = ctx.enter_context(tc.tile_pool(name="psaux", bufs=2, space="PSUM"))
            ps_ctx = ctx.enter_context(tc.tile_pool(name="psctx", bufs=2, space="PSUM"))
            ps_tp = ctx.enter_context(tc.tile_pool(name="pstp", bufs=2, space="PSUM"))

            # ---- persistent tiles ----
            x = [pp.tile([128, H], f32, name=f"x{qt}") for qt in range(NQT)]
            ident = pp.tile([128, 128], fp16, name="ident")
            make_identity(nc, ident[:])
            ones_row = pp.tile([1, 128], fp16, name="ones_row")
            nc.gpsimd.memset(ones_row[:], 1.0)
            ident32 = pp.tile([128, 128], f32, name="ident32")
            nc.vector.tensor_copy(ident32[:], ident[:])
            maskb = pp.tile([128, NQT], f32, name="maskb")
            nc.sync.dma_start(maskb[:], maskd[:])
            v_aug = [pp.tile([128, NH, 65], bf16, name=f"vaug{tt}") for tt in range(NQT)]
            for tt in range(NQT):
                nc.gpsimd.memset(v_aug[tt][:, :, 64:65], 1.0)
            for qt in range(NQT):
                nc.sync.dma_start(x[qt][:], x0d[qt, :, :])

            # rotating engine chooser for elementwise work.
            # GPSIMD (Pool) cannot touch PSUM, so any op with a PSUM operand
            # is restricted to DVE (0) / Act (2).
            _rot = itertools.cycle([0, 1, 2])
            PSUM = bass.MemorySpace.PSUM

            def _no_pool(engs, *aps):
                if any(isinstance(a, bass.AP) and a.space == PSUM for a in aps):
                    engs = tuple(e for e in engs if e != 1) or (0,)
                return engs

            def _pick(engs):
                i = next(_rot)
                while i not in engs:
                    i = next(_rot)
                return i

            def e_copy(out, in_, engs=(0, 1, 2)):
                i = _pick(_no_pool(engs, out, in_))
                if i == 0:
                    nc.vector.tensor_copy(out, in_)
                elif i == 1:
                    nc.gpsimd.tensor_copy(out, in_)
                else:
                    nc.scalar.copy(out, in_)

            def e_scale_bias(out, in_, scale, bias, engs=(0, 1, 2)):
                """out = in_*scale + bias; scale/bias imm or [128,1] AP."""
                i = _pick(_no_pool(engs, out, in_))
                if i == 0:
                    nc.vector.tensor_scalar(out, in_, scale, bias,
                                            op0=ALU.mult, op1=ALU.add)
                elif i == 1:
                    nc.gpsimd.tensor_scalar(out, in_, scale, bias,
                                            op0=ALU.mult, op1=ALU.add)
                else:
                    nc.scalar.activation(out, in_, AF.Identity,
                                         bias=bias, scale=scale)

            def e_mul(out, a, b, engs=(0, 1)):
                i = _pick(_no_pool(engs, out, a, b))
                (nc.vector if i != 1 else nc.gpsimd).tensor_mul(out, a, b)

            def e_add(out, a, b, engs=(0, 1)):
                i = _pick(_no_pool(engs, out, a, b))
                (nc.vector if i != 1 else nc.gpsimd).tensor_add(out, a, b)


            # ---------------- layer-norm helper ----------------
            def ln_finish(stats_ap):
                """aggregate bn_stats columns -> (rstd, negb)."""
                mv = sb.tile([128, 2], f32, tag="lnmv", name="lnmv")
                nc.vector.bn_aggr(mv[:], stats_ap)
                iv = sb.tile([128, 1], f32, tag="lniv", name="lniv", bufs=3)
                nc.vector.tensor_scalar(iv[:], mv[:, 1:2], EPS, None, op0=ALU.add)
                nc.vector.reciprocal(iv[:], iv[:])
                rstd = sb.tile([128, 1], f32, tag="lnrstd", name="lnrstd", bufs=3)
                nc.scalar.sqrt(rstd[:], iv[:])
                negb = sb.tile([128, 1], f32, tag="lnnegb", name="lnnegb", bufs=3)
                nc.vector.scalar_tensor_tensor(
                    negb[:], mv[:, 0:1], -1.0, rstd[:],
                    op0=ALU.mult, op1=ALU.mult)
                return rstd, negb

            def ln_stats(chunks, tag):
                """chunks: list of APs [128, w<=512]; returns (rstd, negb)."""
                nst = len(chunks)
                stats = sb.tile([128, nst * 6], f32, tag="lnstats", name="lnstats")
                for i, cap in enumerate(chunks):
                    nc.vector.bn_stats(stats[:, i * 6:(i + 1) * 6], cap)
                return ln_finish(stats[:])

            def ln_apply(dst, src, rstd, negb, width, engs=(0, 1, 2)):
                """normalize src into dst, chunked so engines rotate."""
                cw = 512 if width % 512 == 0 else width // 2
                for c0 in range(0, width, cw):
                    e_scale_bias(dst[:, c0:c0 + cw], src[:, c0:c0 + cw],
                                 rstd[:], negb[:], engs=engs)

            def ln_chunks(t, width):
                if width == H:
                    return [t[:, 0:384], t[:, 384:768]]
                return [t[:, c * 512:(c + 1) * 512] for c in range(width // 512)]

            def ln_to(dst_tiles, src_tiles, width, tag):
                for qt in range(NQT):
                    rstd, negb = ln_stats(ln_chunks(src_tiles[qt], width), tag)
                    ln_apply(dst_tiles[qt], src_tiles[qt], rstd, negb, width)

            def transpose_h(tiles, nh_tiles, out_tiles):
                """token-major [128, nh*128] tiles -> per-hc [128, 512] (tokens free)."""
                for hc in range(nh_tiles):
                    pt = ps_tp.tile([128, 512], fp16, tag="tp", name="tp")
                    for qt in range(NQT):
                        nc.tensor.transpose(pt[:, qt * 128:(qt + 1) * 128],
                                            tiles[qt][:, hc * 128:(hc + 1) * 128],
                                            ident[:])
                    e_copy(out_tiles[hc][:], pt[:])

            # ---------------- layers ----------------
            hs_cur = None
            for li in range(n_layers):
                par = li % 2
                # ---- attention input LN + transpose ----
                # (for layers > 0 the LN'd hs tiles were produced at the tail
                #  of the previous layer's FFN, overlapping the W2 matmuls)
                if hs_cur is None:
                    hs = [sb.tile([128, H], fp16, tag=f"hs{qt}", name=f"hs{qt}", bufs=1)
                          for qt in range(NQT)]
                    ln_to(hs, x, H, "hsln")
                else:
                    hs = hs_cur
                hsT = [sb.tile([128, 512], fp16, tag=f"xT{hc}", name=f"xT{hc}", bufs=1)
                       for hc in range(NHT)]
                transpose_h(hs, NHT, hsT)

                # ---- QK^T projection: 12 o-tiles [128, 512] (o on partitions) ----
                wqk_sb = wp.tile([128, NHT * 1536], fp16, tag="wqk", name="wqk_sb", bufs=1)
                nc.sync.dma_start(wqk_sb[:], wqk[li, :, :])
                bqk_sb = wp.tile([128, 12], f32, tag="bqk", name="bqk_sb", bufs=1)
                nc.sync.dma_start(bqk_sb[:], bqkd[li, :, :])
                qkT = []
                for ot in range(12):
                    po = ps_mm.tile([128, 512], f32, tag="mm", name="po")
                    for hc in range(NHT):
                        nc.tensor.matmul(
                            po[:], wqk_sb[:, hc * 1536 + ot * 128:hc * 1536 + (ot + 1) * 128],
                            hsT[hc][:], start=(hc == 0), stop=(hc == NHT - 1))
                    o = sb.tile([128, 512], fp16, tag=f"qkT{ot}", name=f"qkT{ot}", bufs=1)
                    sc = SCALE if ot < 6 else 1.0
                    e_scale_bias(o[:], po[:], sc, bqk_sb[:, ot:ot + 1])
                    qkT.append(o)

                # ---- positional C blocks for all heads -> DRAM,
                #      VG projection groups interleaved to fill PE while
                #      the C writes drain on the DMA engines ----
                wvg_sb = wp.tile([128, NHT * 1536], fp16, tag="wvg", name="wvg_sb", bufs=1)
                nc.sync.dma_start(wvg_sb[:], wvg[li, :, :])
                bb_sb = wp.tile([1, 3072], fp16, tag="bb", name="bb_sb", bufs=1)
                nc.sync.dma_start(bb_sb[:], bbd[li, :, :])
                bc_sb = sb.tile([128, 768], fp16, tag="bc", name="bc_sb", bufs=1)
                nc.sync.dma_start(bc_sb[:], bcd[li, :, :])
                g_sb = [sb.tile([128, H], fp16, tag=f"g{tt}", name=f"g{tt}", bufs=1)
                        for tt in range(NQT)]

                gg_sb = [None] * NQT
                pre_reads = {}

                def make_reads(h):
                    c2p = sb.tile([128, NQT, 512], fp16, tag="c2p", name="c2p", bufs=2)
                    nc.sync.dma_start(
                        c2p[:], bass.AP(cd, cd_base(par, 0, h) + 127,
                                        [[WEXP - 1, 128], [CBLK, NQT], [1, 512]]))
                    p2c = sb.tile([128, NQT, 512], fp16, tag="p2c", name="p2c", bufs=2)
                    nc.sync.dma_start(
                        p2c[:], bass.AP(cd, cd_base(par, 1, h) + 127,
                                        [[WEXP - 1, 128], [CBLK, NQT], [1, 512]]))
                    return c2p, p2c

                def vg_group(j):
                    tt, oc = j // 3, j % 3
                    po = ps_mm.tile([128, 512], f32, tag="mm", name="po")
                    for hc in range(NHT):
                        nc.tensor.matmul(
                            po[:], hsT[hc][:, tt * 128:(tt + 1) * 128],
                            wvg_sb[:, hc * 1536 + oc * 512:hc * 1536 + (oc + 1) * 512],
                            start=(hc == 0), stop=False)
                    nc.tensor.matmul(po[:], ones_row[:],
                                     bb_sb[:, oc * 512:(oc + 1) * 512],
                                     start=False, stop=True)
                    if oc == 0:
                        e_copy(v_aug[tt][:, 0:8, 0:64],
                               po[:].rearrange("p (a b) -> p a b", a=8))
                    elif oc == 1:
                        e_copy(v_aug[tt][:, 8:12, 0:64],
                               po[:, 0:256].rearrange("p (a b) -> p a b", a=4))
                        e_copy(g_sb[tt][:, 0:256], po[:, 256:512])
                    else:
                        e_copy(g_sb[tt][:, 256:768], po[:])
                        # g complete: gelu it in place (Act is light in the C
                        # phase); gg_sb aliases g_sb
                        nc.scalar.activation(g_sb[tt][:], g_sb[tt][:], AF.Gelu)
                        gg_sb[tt] = g_sb[tt]

                t12_sb = None
                for h in range(NH):
                    hh, hp = h // 2, (h % 2) * 64
                    if h % 2 == 0:
                        t12_sb = sb.tile([128, 2048], fp16, tag="t12", name="t12_sb", bufs=2)
                        nc.sync.dma_start(t12_sb[:], t12d[li, hh, :, :])
                    qT_h = qkT[h // 2][hp:hp + 64, :]
                    kT_h = qkT[6 + h // 2][hp:hp + 64, :]
                    for tbl, lhs_full in ((0, qT_h), (1, kT_h)):
                        stg = sb.tile([128, NQT * WEXP], fp16, tag="cstg", name="cstg", bufs=2)
                        for bt in range(NQT):
                            j0 = 384 - 128 * bt
                            tb = t12_sb[hp:hp + 64, tbl * 1024:]
                            pa = ps_mm.tile([128, 512], f32, tag="mm", name="pa")
                            nc.tensor.matmul(pa[:], lhs_full[:, bt * 128:(bt + 1) * 128],
                                             tb[:, j0:j0 + 512], start=True, stop=True)
                            pb = ps_aux.tile([128, 128], f32, tag="aux", name="pb")
                            nc.tensor.matmul(pb[:], lhs_full[:, bt * 128:(bt + 1) * 128],
                                             tb[:, j0 + 512:j0 + 640], start=True, stop=True)
                            e_copy(stg[:, bt * WEXP:bt * WEXP + 512], pa[:])
                            e_copy(stg[:, bt * WEXP + 512:(bt + 1) * WEXP], pb[:],
                                   engs=(2,))
                        nc.sync.dma_start(
                            bass.AP(cd, cd_base(par, tbl, h),
                                    [[WEXP, 128], [CBLK, NQT], [1, WEXP]]),
                            stg[:])
                    vg_group(h)
                    if h == 2:
                        # prefetch the first heads' skew reads while the
                        # later C writes drain
                        pre_reads[0] = make_reads(0)
                    elif h == 5:
                        pre_reads[1] = make_reads(1)

                # ---- attention per head ----
                # ctx columns are written pre-gated: ctx * (1/den) * gelu(g)
                ctx_sb = [sb.tile([128, H], fp16, tag=f"ctx{qt}", name=f"ctx{qt}", bufs=1)
                          for qt in range(NQT)]
                gstats = [None] * NQT
                for h in range(NH):
                    if h + 1 < NH and (h + 1) not in pre_reads:
                        pre_reads[h + 1] = make_reads(h + 1)
                    c2p, p2c = pre_reads.pop(h)
                    hp = (h % 2) * 64
                    qT_h = qkT[h // 2][hp:hp + 64, :]
                    kT_h = qkT[6 + h // 2][hp:hp + 64, :]

                    ctxT_ps = ps_ctx.tile([65, 512], f32, tag="ctx", name="ctxT_ps")
                    ss = [None] * NQT
                    rel = [None] * NQT

                    def scores_begin(kt):
                        ps_s = ps_mm.tile([128, 512], f32, tag="mm", name="ps_s")
                        nc.tensor.matmul(ps_s[:], kT_h[:, kt * 128:(kt + 1) * 128],
                                         qT_h[:], start=True, stop=False,
                                         skip_group_check=True)
                        pc2 = ps_tp.tile([128, 512], fp16, tag="tp", name="pc2")
                        for qt in range(NQT):
                            nc.tensor.transpose(pc2[:, qt * 128:(qt + 1) * 128],
                                                c2p[:, qt, kt * 128:(kt + 1) * 128],
                                                ident[:])
                        # rel = c2p^T + p2c in one engine op (psum + sbuf -> sbuf);
                        # positional logits are O(1), fp16 addition is plenty
                        rl = sb.tile([128, 512], fp16, tag="c2pT", name="rel", bufs=2)
                        with nc.allow_low_precision(reason="fp16 add of O(1) rel logits"):
                            e_add(rl[:], pc2[:], p2c[:, kt, :])
                        ss[kt] = ps_s
                        rel[kt] = rl

                    def scores_finish(kt):
                        ps_s = ss[kt]
                        nc.tensor.matmul(ps_s[:], ident[:], rel[kt][:],
                                         start=False, stop=True,
                                         skip_group_check=True)
                        pT = sb.tile([128, 512], bf16, tag="pT", name="pT", bufs=3)
                        nc.scalar.activation(pT[:], ps_s[:], AF.Exp,
                                             bias=maskb[:, kt:kt + 1])
                        nc.tensor.matmul(ctxT_ps[:], v_aug[kt][:, h, :], pT[:],
                                         start=(kt == 0), stop=(kt == NQT - 1),
                                         skip_group_check=True)

                    for kt in range(NQT):
                        scores_begin(kt)
                        if kt > 0:
                            scores_finish(kt - 1)
                    scores_finish(NQT - 1)

                    # ctx^T [65, 512] -> per-q-tile [128, 64] with 1/den scaling;
                    # the denominator row is reciprocal'd during the copy so the
                    # transposed tile carries 1/den in column 64 directly.
                    ctxT_sb = sb.tile([65, 512], fp16, tag="ctxTsb", name="ctxTsb", bufs=2)
                    e_copy(ctxT_sb[:], ctxT_ps[:])
                    for qt in range(NQT):
                        pc = ps_tp.tile([128, 65], fp16, tag="tp", name="pc")
                        nc.tensor.transpose(pc[:], ctxT_sb[:, qt * 128:(qt + 1) * 128],
                                            ident[:65, :65])
                        dinv = sb.tile([128, 1], f32, tag="dinv", name="dinv", bufs=3)
                        nc.vector.reciprocal(dinv[:], pc[:, 64:65])
                        # fused (ctx * 1/den) * gelu(g) -> pre-gated context
                        nc.vector.scalar_tensor_tensor(
                            ctx_sb[qt][:, h * 64:(h + 1) * 64],
                            pc[:, 0:64], dinv[:],
                            gg_sb[qt][:, h * 64:(h + 1) * 64],
                            op0=ALU.mult, op1=ALU.mult)
                    # incremental gate-LN stats: half of ctx's columns become
                    # final after head 5, the rest after head 11
                    if h == 5 or h == NH - 1:
                        c0 = 0 if h == 5 else 384
                        s0 = 0 if h == 5 else 6
                        for qt in range(NQT):
                            if h == 5:
                                gstats[qt] = sb.tile([128, 12], f32, tag=f"gst{qt}",
                                                     name=f"gst{qt}", bufs=1)
                            nc.vector.bn_stats(gstats[qt][:, s0:s0 + 6],
                                               ctx_sb[qt][:, c0:c0 + 384])

                # ---- deferred-LN out proj ----
                # LN of the gated ctx commutes through the linear Wout:
                #   Wout @ (rstd*(cg - mean)) = rstd*(Wout@cg) - (mean*rstd)*rowsum(Wout)
                # so the transposes consume RAW gated ctx and the scale /
                # rank-1 correction / bout ride on the residual update.
                wout_sb = wp.tile([128, NHT * 768], fp16, tag="wout", name="wout_sb", bufs=1)
                nc.sync.dma_start(wout_sb[:], wout[li, :, :])
                rstd_cg = [None] * NQT
                mr_cg = [None] * NQT
                for qt in range(NQT):
                    mv = sb.tile([128, 2], f32, tag="gmv", name="gmv", bufs=4)
                    nc.vector.bn_aggr(mv[:], gstats[qt][:])
                    iv = sb.tile([128, 1], f32, tag="giv", name="giv", bufs=4)
                    nc.vector.tensor_scalar(iv[:], mv[:, 1:2], EPS, None, op0=ALU.add)
                    nc.vector.reciprocal(iv[:], iv[:])
                    r = sb.tile([128, 1], f32, tag=f"grstd{qt}", name=f"grstd{qt}", bufs=1)
                    nc.scalar.sqrt(r[:], iv[:])
                    pm = ps_tp.tile([1, 128], f32, tag="tp", name="pm")
                    nc.tensor.transpose(pm[:], mv[:, 0:1], ident32[:])
                    m = sb.tile([1, 128], fp16, tag=f"gmr{qt}", name=f"gmr{qt}", bufs=1)
                    e_copy(m[:], pm[:])
                    rstd_cg[qt] = r
                    mr_cg[qt] = m
                cgT = [sb.tile([128, 512], fp16, tag=f"xT{hc}", name=f"cgT{hc}", bufs=1)
                       for hc in range(NHT)]
                transpose_h(ctx_sb, NHT, cgT)
                h2 = []
                for qt in range(NQT):
                    for oc in range(2):
                        w = 512 if oc == 0 else 256
                        pool = ps_mm if oc == 0 else ps_aux
                        po = pool.tile([128, w], f32, tag="mm" if oc == 0 else "aux",
                                       name="po")
                        for hc in range(NHT):
                            nc.tensor.matmul(
                                po[:], cgT[hc][:, qt * 128:(qt + 1) * 128],
                                wout_sb[:, hc * 768 + oc * 512:hc * 768 + oc * 512 + w],
                                start=(hc == 0), stop=False)
                        nc.tensor.matmul(
                            po[:], mr_cg[qt][:],
                            bb_sb[:, 1536 + oc * 512:1536 + oc * 512 + w],
                            start=False, stop=True)
                        sl = x[qt][:, oc * 512:oc * 512 + w]
                        nc.vector.scalar_tensor_tensor(
                            sl, po[:], rstd_cg[qt][:], sl, op0=ALU.mult, op1=ALU.add)
                    # + bout (broadcast row; zero for this model's inputs)
                    e_add(x[qt][:], x[qt][:], bc_sb[:, 0:768])
                    # FFN input LN for this qt, overlapping later out-proj work
                    h2_t = sb.tile([128, H], fp16, tag=f"hs{qt}", name=f"h2{qt}", bufs=1)
                    rstd, negb = ln_stats(ln_chunks(x[qt], H), "h2ln")
                    ln_apply(h2_t, x[qt], rstd, negb, H, engs=(1, 2))
                    h2.append(h2_t)

                # ---- FFN: single W1 pass, W2 in two qt-half passes ----
                h2T = [sb.tile([128, 512], fp16, tag=f"xT{hc}", name=f"h2T{hc}", bufs=1)
                       for hc in range(NHT)]
                transpose_h(h2, NHT, h2T)
                u = [sb.tile([128, FI], fp16, tag=f"u{qt}", name=f"u{qt}", bufs=1)
                     for qt in range(NQT)]
                ustats = [sb.tile([128, 24], f32, tag=f"ust{qt}", name=f"ust{qt}",
                                  bufs=1) for qt in range(NQT)]
                for oc in range(8):
                    w1_sb = wp.tile([128, NHT * 512], fp16, tag="w1",
                                    name="w1_sb", bufs=3)
                    nc.sync.dma_start(w1_sb[:], w1[li, oc, :, :])
                    for qt in range(NQT):
                        po = ps_mm.tile([128, 512], f32, tag="mm", name="po")
                        for hc in range(NHT):
                            nc.tensor.matmul(
                                po[:], h2T[hc][:, qt * 128:(qt + 1) * 128],
                                w1_sb[:, hc * 512:(hc + 1) * 512],
                                start=(hc == 0), stop=(hc == NHT - 1))
                        if oc < 4:
                            e_copy(u[qt][:, oc * 512:(oc + 1) * 512], po[:])
                        else:
                            gt = sb.tile([128, 512], fp16, tag="ffng",
                                         name="ffng", bufs=2)
                            nc.scalar.activation(gt[:], po[:], AF.Gelu_apprx_tanh)
                            lo = (oc - 4) * 512
                            e_mul(u[qt][:, lo:lo + 512], u[qt][:, lo:lo + 512],
                                  gt[:])
                            # this 512-chunk of u is final: stats now
                            cix = oc - 4
                            nc.vector.bn_stats(ustats[qt][:, cix * 6:(cix + 1) * 6],
                                               u[qt][:, lo:lo + 512])
                # deferred LN(u): W2 is linear (and bias-free), so feed RAW u
                # into the transposes and apply rstd / the mean correction at
                # the residual update.
                rstd_u = [None] * NQT
                mr_u = [None] * NQT
                for qt in range(NQT):
                    mv = sb.tile([128, 2], f32, tag="umv", name="umv", bufs=4)
                    nc.vector.bn_aggr(mv[:], ustats[qt][:])
                    iv = sb.tile([128, 1], f32, tag="uiv", name="uiv", bufs=4)
                    nc.vector.tensor_scalar(iv[:], mv[:, 1:2], EPS, None, op0=ALU.add)
                    nc.vector.reciprocal(iv[:], iv[:])
                    r = sb.tile([128, 1], f32, tag=f"urstd{qt}", name=f"urstd{qt}", bufs=1)
                    nc.scalar.sqrt(r[:], iv[:])
                    pm = ps_tp.tile([1, 128], f32, tag="tp", name="pm")
                    nc.tensor.transpose(pm[:], mv[:, 0:1], ident32[:])
                    m = sb.tile([1, 128], fp16, tag=f"umr{qt}", name=f"umr{qt}", bufs=1)
                    e_copy(m[:], pm[:])
                    rstd_u[qt] = r
                    mr_u[qt] = m
                hs_next = [None] * NQT
                for half in range(2):
                    unT = [sb.tile([128, 256], fp16, tag=f"unT{ic}",
                                   name=f"unT{ic}", bufs=1) for ic in range(16)]
                    for ic in range(16):
                        pt = ps_tp.tile([128, 256], fp16, tag="tp", name="pt")
                        for qi in range(2):
                            nc.tensor.transpose(pt[:, qi * 128:(qi + 1) * 128],
                                                u[half * 2 + qi][:, ic * 128:(ic + 1) * 128],
                                                ident[:])
                        e_copy(unT[ic][:], pt[:])
                    pos = [None, None]
                    for icp in range(8):
                        w2_sb = wp.tile([128, 1536], fp16, tag="w2", name="w2_sb", bufs=3)
                        nc.sync.dma_start(w2_sb[:], w2[li, icp, :, :])
                        for qi in range(2):
                            if icp == 0:
                                pos[qi] = (
                                    ps_mm.tile([128, 512], f32, tag="mm", name="po"),
                                    ps_aux.tile([128, 256], f32, tag="aux", name="po2"))
                            for i2 in range(2):
                                ic = icp * 2 + i2
                                for oc in range(2):
                                    w = 512 if oc == 0 else 256
                                    nc.tensor.matmul(
                                        pos[qi][oc][:],
                                        unT[ic][:, qi * 128:(qi + 1) * 128],
                                        w2_sb[:, i2 * 768 + oc * 512:i2 * 768 + oc * 512 + w],
                                        start=(icp == 0 and i2 == 0),
                                        stop=False,
                                        skip_group_check=True)
                    for qi in range(2):
                        qt = half * 2 + qi
                        for oc in range(2):
                            w = 512 if oc == 0 else 256
                            nc.tensor.matmul(
                                pos[qi][oc][:], mr_u[qt][:],
                                bb_sb[:, 2304 + oc * 512:2304 + oc * 512 + w],
                                start=False, stop=True, skip_group_check=True)
                            sl = x[qt][:, oc * 512:oc * 512 + w]
                            nc.vector.scalar_tensor_tensor(
                                sl, pos[qi][oc][:], rstd_u[qt][:], sl,
                                op0=ALU.mult, op1=ALU.add)
                        if li < n_layers - 1:
                            # next layer's attention-input LN, overlapping the
                            # other FFN half's matmuls
                            t = sb.tile([128, H], fp16, tag=f"hs{qt}",
                                        name=f"hsn{qt}", bufs=1)
                            rstd, negb = ln_stats(ln_chunks(x[qt], H), "hsln")
                            ln_apply(t, x[qt], rstd, negb, H, engs=(1, 2))
                            hs_next[qt] = t
                hs_cur = hs_next if li < n_layers - 1 else None

            # ---- output ----
            for qt in range(NQT):
                nc.sync.dma_start(yd[qt, :, :], x[qt][:])

    nc.finalize()
    return nc


_CACHE = {}


def _get_nc(n_layers):
    if n_layers not in _CACHE:
        _CACHE[n_layers] = _build(n_layers)
    return _CACHE[n_layers]


# ---------------------------------------------------------------- host prep
def _prep_shared(word_emb, rel_emb, rel_g, rel_b, Wqk, bqk, Wvg, bvg, Wout,
                 bout, W1, W2, n_layers):
    beta = _beta_delta()                     # [1023]
    idx_c2p = beta[1022 - np.arange(1023)]   # T1: delta = 511 - j
    idx_p2c = beta[np.arange(1023)]          # T2: delta = j - 511
    rel = _ln_np(rel_emb.astype(np.float64)).astype(np.float32) * rel_g + rel_b

    d = {}
    t12 = np.zeros((n_layers, NH, 64, 2048), np.float32)
    wqk_t = np.zeros((n_layers, 128, NHT * 1536), np.float32)
    wvg_t = np.zeros((n_layers, 128, NHT * 1536), np.float32)
    wout_t = np.zeros((n_layers, 128, NHT * 768), np.float32)
    w1_t = np.zeros((n_layers, 8, 128, NHT * 512), np.float32)
    w2_t = np.zeros((n_layers, 8, 128, 2 * 768), np.float32)
    bqk_t = np.zeros((n_layers, 128, 12), np.float32)
    bb_t = np.zeros((n_layers, 1, 3072), np.float32)
    bc_t = np.zeros((n_layers, 1, 768), np.float32)
    for li in range(n_layers):
        pos = rel @ Wqk[li].T + bqk[li]          # [63, 1536]
        qpos = pos[:, :H].reshape(63, NH, 64)
        kpos = pos[:, H:].reshape(63, NH, 64)
        # T1[j] = kpos[beta(511-j)], T2[j] = qpos[beta(j-511)] * SCALE
        t12[li, :, :, :1023] = kpos[idx_c2p].transpose(1, 2, 0)
        t12[li, :, :, 1024:2047] = qpos[idx_p2c].transpose(1, 2, 0) * SCALE

        wqkT = Wqk[li].T.copy()                  # [768, 1536]
        wqk_t[li] = wqkT.reshape(NHT, 128, 1536).transpose(1, 0, 2).reshape(128, -1)
        wvg_t[li] = Wvg[li].T.reshape(NHT, 128, 1536).transpose(1, 0, 2).reshape(128, -1)
        wout_t[li] = Wout[li].T.reshape(NHT, 128, 768).transpose(1, 0, 2).reshape(128, -1)
        w1T = W1[li].T                           # [768, 4096]
        for oc in range(8):
            w1_t[li, oc] = (w1T[:, oc * 512:(oc + 1) * 512]
                            .reshape(NHT, 128, 512).transpose(1, 0, 2).reshape(128, -1))
        w2T = W2[li].T                           # [2048, 768]
        for icp in range(8):
            w2_t[li, icp] = (w2T[icp * 256:(icp + 1) * 256, :]
                             .reshape(2, 128, 768).transpose(1, 0, 2).reshape(128, -1))
        bqk_t[li] = bqk[li].reshape(12, 128).T
        bb_t[li, 0, :1536] = bvg[li]
        bb_t[li, 0, 1536:2304] = -Wout[li].sum(axis=1)
        bb_t[li, 0, 2304:3072] = -W2[li].sum(axis=1)
        bc_t[li, 0, 0:768] = bout[li]

    d["wqk"] = wqk_t.astype(F16)
    d["wvg"] = wvg_t.astype(F16)
    d["wout"] = wout_t.astype(F16)
    d["w1"] = w1_t.astype(F16)
    d["w2"] = w2_t.astype(F16)
    # [L, NH, 64, 2048] -> [L, NH//2, 128, 2048]
    d["t12d"] = t12.reshape(n_layers, NH // 2, 128, 2048).astype(F16)
    d["bqkd"] = bqk_t
    d["bbd"] = bb_t.astype(F16)
    d["bcd"] = np.broadcast_to(bc_t.astype(F16), (n_layers, 128, 768)).copy()
    return d


def _make_in_maps(inputs, n_layers):
    input_ids = np.asarray(inputs["input_ids"])
    attention_mask = np.asarray(inputs["attention_mask"])
    word_emb = np.asarray(inputs["word_emb"], np.float32)

    shared = _prep_shared(
        word_emb, np.asarray(inputs["rel_emb"], np.float32),
        np.asarray(inputs["rel_g"], np.float32), np.asarray(inputs["rel_b"], np.float32),
        np.asarray(inputs["Wqk"], np.float32), np.asarray(inputs["bqk"], np.float32),
        np.asarray(inputs["Wvg"], np.float32), np.asarray(inputs["bvg"], np.float32),
        np.asarray(inputs["Wout"], np.float32), np.asarray(inputs["bout"], np.float32),
        np.asarray(inputs["W1"], np.float32), np.asarray(inputs["W2"], np.float32),
        n_layers)

    in_maps = []
    for b in range(B):
        m = dict(shared)
        x0 = _ln_np(word_emb[input_ids[:, b]].astype(np.float64)).astype(np.float32)
        m["x0d"] = x0.reshape(NQT, 128, H)
        mb = np.where(attention_mask[b, 0, 0, :], MASK_NEG, 0.0).astype(np.float32)
        m["maskd"] = mb.reshape(NQT, 128).T.copy()
        in_maps.append(m)
    return in_maps


def run(inputs, n_layers=L, trace=False):
    nc = _get_nc(n_layers)
    in_maps = _make_in_maps(inputs, n_layers)
    res = run_bass_kernel_spmd(nc, in_maps, core_ids=list(range(B)), trace=trace)
    out = np.zeros((S, B, H), np.float32)
    for b in range(B):
        out[:, b, :] = res.results[b]["yd"].reshape(S, H)
    return out, res


def kernel(**inputs) -> np.ndarray:
    out, _ = run(inputs, L)
    return out


# ------------------------------------------------------- timing-only runner
def make_timed_runner(n_layers, in_maps):
    """Build a persistent jitted PJRT callable over 8 cores for wall-clock
    timing (the axon NTFF profile hook is unavailable in this container)."""
    import jax
    from jax.sharding import Mesh, PartitionSpec, NamedSharding
    from jax.experimental.shard_map import shard_map
    from concourse import bass2jax

    nc = _get_nc(n_layers)
    bass2jax.install_neuronx_cc_hook()
    partition_name = nc.partition_id_tensor.name if nc.partition_id_tensor else None
    in_names, out_names, out_avals, zero_outs = [], [], [], []
    import concourse.mybir as _mb
    for alloc in nc.m.functions[0].allocations:
        if not isinstance(alloc, _mb.MemoryLocationSet):
            continue
        name = alloc.memorylocations[0].name
        if alloc.kind == "ExternalInput":
            if name != partition_name:
                in_names.append(name)
        elif alloc.kind == "ExternalOutput":
            out_names.append(name)
            shape = tuple(alloc.tensor_shape)
            dtype = _mb.dt.np(alloc.dtype)
            out_avals.append(jax.core.ShapedArray(shape, dtype))
            zero_outs.append(np.zeros(shape, dtype))
    n_params = len(in_names)
    n_outs = len(out_avals)
    all_in_names = list(in_names) + out_names
    if partition_name is not None:
        all_in_names = all_in_names + [partition_name]

    def _body(*args):
        operands = list(args)
        if partition_name is not None:
            operands.append(bass2jax.partition_id_tensor())
        outs = bass2jax._bass_exec_p.bind(
            *operands, out_avals=tuple(out_avals), in_names=tuple(all_in_names),
            out_names=tuple(out_names), lowering_input_output_aliases=(),
            sim_require_finite=True, sim_require_nnan=True, nc=nc)
        return tuple(outs)

    n_cores = B
    devices = jax.devices()[:n_cores]
    mesh = Mesh(np.asarray(devices), ("core",))
    P = PartitionSpec
    sharded = jax.jit(
        shard_map(_body, mesh=mesh, in_specs=(P("core"),) * (n_params + n_outs),
                  out_specs=(P("core"),) * n_outs, check_rep=False),
        keep_unused=True)

    concat_in = [
        np.concatenate([np.asarray(in_maps[c][nm]) for c in range(n_cores)], axis=0)
        for nm in in_names]
    concat_zeros = [np.zeros((n_cores * z.shape[0], *z.shape[1:]), z.dtype)
                    for z in zero_outs]
    shard = NamedSharding(mesh, P("core"))
    dev_in = [jax.device_put(a, shard) for a in concat_in]
    dev_zeros = [jax.device_put(a, shard) for a in concat_zeros]

    def call():
        outs = sharded(*dev_in, *dev_zeros)
        jax.block_until_ready(outs)
        return outs

    return call
